# revision 25
# baseline (speedup 1.0000x reference)
"""BSI quantized linear kernel for Trainium2 (8 NeuronCores, SPMD).

Computes out = round(x*100)/100 @ (round(W*100)/100).T + b for
x [4096, 4096] f32, W [4096, 4096] f32, b [4096] f32.

Sharding: 2x4 grid. x is sharded over the token dim into 2 halves;
W and b over out_features into 4 quarters. Core (r, c) = id r*4+c
computes out[r*2048:(r+1)*2048, c*1024:(c+1)*1024]; the host
assembles the 8 blocks.

Inputs are handed to each core pre-transposed (x^T and W^T slices —
host-side numpy layout change only, values bit-identical), so both
matmul operands arrive with the contraction dim on partitions and the
PE does no transposes: its only work is 1024 chained 128x512 fp16
matmuls (216 ns issue-to-issue measured => ~221 us at full rate).

Quantization (exact): DVE computes fl32(100*v) + 1.5*2^23 (f32 magic
constant => round-half-to-even onto the integer grid, matching
jnp.round bitwise), ACT subtracts the magic and emits fp16 (integers
|.|<=~550 are exact in fp16). PSUM accumulates exact integer dots in
f32; epilogue computes 1e-4*psum + bias in one DVE op per o-half.

Scheduling notes (from traces of earlier revisions):
  - Small per-kt DMAs serialize on HWDGE trigger processing (~230
    GB/s effective): load multi-kt chunks with one trigger each.
  - 3D SBUF tiles cost ~2x on ACT and +15% on matmul operands: all
    compute tiles are 2D; only the DMA *access patterns* are 3D
    (dst tile viewed p (k o) -> p k o, src DRAM (k p) o -> p k o).
  - TRN2 has two HWDGE queues (Sync and Scalar/Activation): input
    chunks alternate between them so both stream concurrently during
    the W+band-0 ramp (a single queue toped out at ~290-340 GB/s).
  - The first chunks are 1 k-tile so the DMA->DVE->ACT pipeline fills
    fast and the first matmul starts ~10 us earlier.
  - Output DMAs also go on the Scalar queue (they trail the ramp).
  - Band 0 issues matmuls kt-outer across all 8 PSUM banks to ride
    the DMA ramp; bands 1-3 are prefetched and run chain-sequential
    with per-stripe epilogues; band-0 epilogues issue before the
    band-2 prefetch so PSUM banks recycle promptly on DVE.
"""

import numpy as np

_B, _D, _DOUT = 4096, 4096, 4096
_NCORES = 8
_R, _C = 2, 4              # grid: 2 token-halves x 4 out_feature-quarters
_BPER = _B // _R           # 2048 tokens per core
_OPER = _DOUT // _C        # 1024 out features per core
_MAGIC = 12582912.0        # 1.5 * 2**23
_P = 128

_nc_cache = {}

# Chunk layouts (k-tiles per chunk). First chunks are small to fill the
# quantize pipeline fast; later chunks are fat to amortize triggers.
_WCHUNKS = [1, 1, 2] + [4] * 7           # 32 k-tiles, W chunks
_XCHUNKS0 = [1, 1, 2] + [4] * 7          # 32 k-tiles, x band 0
_XCHUNKS = [4] * 8                       # 32 k-tiles, x bands 1-3


def _starts(chunks):
    s, out = 0, []
    for c in chunks:
        out.append(s)
        s += c
    return out


def _build(BPER, D, OPER):
    import concourse.mybir as mybir
    import concourse.tile as tile
    from concourse import bacc

    f32 = mybir.dt.float32
    f16 = mybir.dt.float16
    P = _P
    KT = D // P            # 32 contraction k-tiles
    NB = 4                 # x bands
    BBAND = BPER // NB     # 512 tokens per band
    SBT = BBAND // P       # 4 stripes per band
    NH = OPER // 512       # 2 o-halves (moving dim is 512 max)
    mult = mybir.AluOpType.mult
    add = mybir.AluOpType.add

    wstarts = _starts(_WCHUNKS)
    xstarts0 = _starts(_XCHUNKS0)
    xstarts = _starts(_XCHUNKS)
    # kt -> (chunk index, offset) maps
    wmap = {}
    for ci, (st0, cnt) in enumerate(zip(wstarts, _WCHUNKS)):
        for k in range(cnt):
            wmap[st0 + k] = (ci, k)
    xmap0, xmap = {}, {}
    for ci, (st0, cnt) in enumerate(zip(xstarts0, _XCHUNKS0)):
        for k in range(cnt):
            xmap0[st0 + k] = (ci, k)
    for ci, (st0, cnt) in enumerate(zip(xstarts, _XCHUNKS)):
        for k in range(cnt):
            xmap[st0 + k] = (ci, k)

    nc = bacc.Bacc("TRN2", target_bir_lowering=False, debug=False,
                   num_devices=_NCORES)
    xt_d = nc.dram_tensor("xt", [D, BPER], f32, kind="ExternalInput").ap()
    wt_d = nc.dram_tensor("wt", [D, OPER], f32, kind="ExternalInput").ap()
    b_d = nc.dram_tensor("b", [OPER], f32, kind="ExternalInput").ap()
    o_d = nc.dram_tensor("out", [BPER, OPER], f32, kind="ExternalOutput").ap()

    with tile.TileContext(nc) as tc:
        with (
            tc.tile_pool(name="const", bufs=1) as cpool,
            tc.tile_pool(name="wq", bufs=1) as wqpool,
            tc.tile_pool(name="xq", bufs=1) as xq0pool,
            tc.tile_pool(name="xqs", bufs=2) as xqpool,
            tc.tile_pool(name="wstg", bufs=2) as wstgpool,
            tc.tile_pool(name="xstg", bufs=2) as xstgpool,
            tc.tile_pool(name="mm", bufs=8, space="PSUM") as mmpool,
            tc.tile_pool(name="osb", bufs=3) as opool,
        ):
            bias_bc = cpool.tile([P, OPER], f32)
            # GpSimd software DGE: keeps the slow broadcast expansion
            # off the Sync HWDGE ring head (it delayed the first input
            # transfer by several us).
            nc.gpsimd.dma_start(bias_bc, b_d[None, :].to_broadcast((P, OPER)))

            # Stripe each input stream across the two HWDGE queues with
            # independent toggles (W starts on Sync, x on Scalar). A
            # single shared toggle correlates with the W/x call
            # alternation and degenerates to W-on-Sync / x-on-Scalar,
            # serializing each stream on one queue.
            _qtog = {"w": 0, "x": 1}

            def dma_in(stream, dst_ap, src_ap):
                eng = nc.sync if _qtog[stream] == 0 else nc.scalar
                _qtog[stream] ^= 1
                eng.dma_start(dst_ap, src_ap)

            wq = [wqpool.tile([P, cnt * OPER], f16, tag=f"wq{c}",
                              name=f"wq{c}")
                  for c, cnt in enumerate(_WCHUNKS)]

            def quant(dst16, src32):
                # fl32(fl32(100*v) + MAGIC) then -MAGIC, both on DVE
                # (rounds half-to-even onto the integer grid, emits
                # fp16). Keeping quantization entirely on DVE leaves the
                # Scalar engine as a pure DMA-trigger queue: an ACT op
                # waiting on DMA data would head-of-line block the
                # Scalar HWDGE ring (measured: x stream serialized
                # behind W quantization).
                nc.vector.tensor_scalar(src32, src32, 100.0, _MAGIC,
                                        mult, add)
                # fp16 convert on GpSimd (SBUF->SBUF): DVE saturates if
                # it carries both quant passes (~143 us), delaying late
                # bands' tiles and backing up the staging rotation.
                nc.gpsimd.tensor_scalar_add(dst16, src32, -_MAGIC)

            def load_w_chunk(c):
                cnt = _WCHUNKS[c]
                k0 = wstarts[c]
                st_full = wstgpool.tile([P, 4 * OPER], f32, tag="wst",
                                        name=f"wst{c}")
                st = st_full[:, :cnt * OPER]
                src = wt_d[k0 * P:(k0 + cnt) * P, :].rearrange(
                    "(k p) o -> p k o", p=P)
                dma_in("w", st.rearrange("p (k o) -> p k o", k=cnt), src)
                quant(wq[c], st)

            def load_x_chunk(band, c, cnt, k0, tiles):
                st_full = xstgpool.tile([P, 4 * BBAND], f32, tag="xst",
                                        name=f"xst{band}_{c}")
                st = st_full[:, :cnt * BBAND]
                src = xt_d[k0 * P:(k0 + cnt) * P,
                           band * BBAND:(band + 1) * BBAND].rearrange(
                    "(k p) b -> p k b", p=P)
                dma_in("x", st.rearrange("p (k b) -> p k b", k=cnt), src)
                if band == 0 and cnt != 4:
                    # Small head chunks get a tiny dedicated pool; band
                    # 0's fat chunks share the rotating xq tags (band 2
                    # reuses them after band 0's matmuls complete).
                    t = xq0pool.tile([P, cnt * BBAND], f16, tag=f"xq0_{c}",
                                     name=f"xq0_{c}")
                else:
                    tg = c - 3 if band == 0 else c
                    t = xqpool.tile([P, cnt * BBAND], f16, tag=f"xq{tg}",
                                    name=f"xq{band}_{c}")
                quant(t, st)
                tiles.append(t)

            def load_band(band):
                tiles = []
                for c, cnt in enumerate(_XCHUNKS):
                    load_x_chunk(band, c, cnt, xstarts[c], tiles)
                return tiles

            def mm(ps, xqt, xm, kt, bt, oh, start, stop):
                xc, xk = xm[kt]
                wc, wk = wmap[kt]
                xof = xk * BBAND + bt * P
                wof = wk * OPER + oh * 512
                nc.tensor.matmul(
                    ps,
                    xqt[xc][:, xof:xof + P],
                    wq[wc][:, wof:wof + 512],
                    start=start, stop=stop)

            def epilogue_bt(band, bt, ps_pair):
                ob = opool.tile([P, OPER], f32, tag="ob",
                                name=f"ob{band}_{bt}")
                for oh in range(NH):
                    nc.vector.scalar_tensor_tensor(
                        ob[:, oh * 512:(oh + 1) * 512], ps_pair[oh], 1e-4,
                        bias_bc[:, oh * 512:(oh + 1) * 512], mult, add)
                row = (band * SBT + bt) * P
                nc.scalar.dma_start(o_d[row:row + P, :], ob)

            # Ramp: issue W chunk c and band-0 x chunk c back-to-back
            # (they cover the same k-tiles), alternating HWDGE queues.
            xq_tiles = [None] * NB
            xq_tiles[0] = []
            for c in range(len(_WCHUNKS)):
                load_w_chunk(c)
                load_x_chunk(0, c, _XCHUNKS0[c], xstarts0[c], xq_tiles[0])
            xq_tiles[1] = load_band(1)

            chains = [(bt, oh) for bt in range(SBT) for oh in range(NH)]

            # Band 0: kt-outer across all 8 PSUM banks (DMA-paced ramp).
            ps0 = [mmpool.tile([P, 512], f32, tag="ps", name=f"ps0_{j}")
                   for j in range(len(chains))]
            for kt in range(KT):
                for j, (bt, oh) in enumerate(chains):
                    mm(ps0[j], xq_tiles[0], xmap0, kt, bt, oh,
                       start=(kt == 0), stop=(kt == KT - 1))
            for bt in range(SBT):
                epilogue_bt(0, bt, ps0[bt * NH:(bt + 1) * NH])
            xq_tiles[2] = load_band(2)

            # Band 1 kt-outer too: its x chunks are still streaming in
            # behind W + band 0 when it starts, and chain-sequential
            # order stalled on the last chunks (measured ~8 us).
            ps1 = [mmpool.tile([P, 512], f32, tag="ps", name=f"ps1_{j}")
                   for j in range(len(chains))]
            for kt in range(KT):
                for j, (bt, oh) in enumerate(chains):
                    mm(ps1[j], xq_tiles[1], xmap, kt, bt, oh,
                       start=(kt == 0), stop=(kt == KT - 1))
            for bt in range(SBT):
                epilogue_bt(1, bt, ps1[bt * NH:(bt + 1) * NH])
            xq_tiles[3] = load_band(3)

            # Bands 2-3: fully prefetched, chain-sequential with
            # per-stripe epilogues (staggers the tail).
            for band in range(2, NB):
                for bt in range(SBT):
                    ps_pair = []
                    for oh in range(NH):
                        ps = mmpool.tile([P, 512], f32, tag="ps",
                                         name=f"ps{band}_{bt}_{oh}")
                        for kt in range(KT):
                            mm(ps, xq_tiles[band], xmap, kt, bt, oh,
                               start=(kt == 0), stop=(kt == KT - 1))
                        ps_pair.append(ps)
                    epilogue_bt(band, bt, ps_pair)

    nc.compile()
    return nc


def _get_nc(BPER=_BPER, D=_D, OPER=_OPER):
    key = (BPER, D, OPER)
    if key not in _nc_cache:
        _nc_cache[key] = _build(BPER, D, OPER)
    return _nc_cache[key]


def _make_in_maps(x, W, b):
    xt = [np.ascontiguousarray(x[r * _BPER:(r + 1) * _BPER, :].T)
          for r in range(_R)]
    wt = [np.ascontiguousarray(W[c * _OPER:(c + 1) * _OPER, :].T)
          for c in range(_C)]
    bs = [np.ascontiguousarray(b[c * _OPER:(c + 1) * _OPER])
          for c in range(_C)]
    in_maps = []
    for r in range(_R):
        for c in range(_C):
            in_maps.append({"xt": xt[r], "wt": wt[c], "b": bs[c]})
    return in_maps


def _assemble(blocks):
    return np.block([[blocks[r * _C + c] for c in range(_C)]
                     for r in range(_R)])


def _run(x, W, b, trace=False):
    from concourse.bass_utils import run_bass_kernel_spmd

    nc = _get_nc()
    in_maps = _make_in_maps(x, W, b)
    res = run_bass_kernel_spmd(nc, in_maps, core_ids=list(range(_NCORES)),
                               trace=trace)
    out = _assemble([res.results[c]["out"] for c in range(_NCORES)])
    return out, res


def kernel(x=None, W=None, b=None):
    x = np.ascontiguousarray(np.asarray(x, dtype=np.float32))
    W = np.ascontiguousarray(np.asarray(W, dtype=np.float32))
    b = np.ascontiguousarray(np.asarray(b, dtype=np.float32))
    out, _ = _run(x, W, b, trace=False)
    return out


# revision 27
# speedup vs baseline: 4.9452x; 4.9452x over previous
"""BSI quantized linear kernel for Trainium2 (8 NeuronCores, SPMD).

Computes out = round(x*100)/100 @ (round(W*100)/100).T + b for
x [4096, 4096] f32, W [4096, 4096] f32, b [4096] f32.

Sharding: 2x4 grid. x is sharded over the token dim into 2 halves;
W and b over out_features into 4 quarters. Core (r, c) = id r*4+c
computes out[r*2048:(r+1)*2048, c*1024:(c+1)*1024]; the host
assembles the 8 blocks.

Inputs are handed to each core pre-transposed (x^T and W^T slices —
host-side numpy layout change only, values bit-identical), so both
matmul operands arrive with the contraction dim on partitions and the
PE does no transposes: its only work is 1024 chained 128x512 fp16
matmuls (216 ns issue-to-issue measured => ~221 us at full rate).

Quantization (exact): DVE computes fl32(100*v) + 1.5*2^23 (f32 magic
constant => round-half-to-even onto the integer grid, matching
jnp.round bitwise), ACT subtracts the magic and emits fp16 (integers
|.|<=~550 are exact in fp16). PSUM accumulates exact integer dots in
f32; epilogue computes 1e-4*psum + bias in one DVE op per o-half.

Scheduling notes (from traces of earlier revisions):
  - Small per-kt DMAs serialize on HWDGE trigger processing (~230
    GB/s effective): load multi-kt chunks with one trigger each.
  - 3D SBUF tiles cost ~2x on ACT and +15% on matmul operands: all
    compute tiles are 2D; only the DMA *access patterns* are 3D
    (dst tile viewed p (k o) -> p k o, src DRAM (k p) o -> p k o).
  - TRN2 has two HWDGE queues (Sync and Scalar/Activation): input
    chunks alternate between them so both stream concurrently during
    the W+band-0 ramp (a single queue toped out at ~290-340 GB/s).
  - The first chunks are 1 k-tile so the DMA->DVE->ACT pipeline fills
    fast and the first matmul starts ~10 us earlier.
  - Output DMAs also go on the Scalar queue (they trail the ramp).
  - Band 0 issues matmuls kt-outer across all 8 PSUM banks to ride
    the DMA ramp; bands 1-3 are prefetched and run chain-sequential
    with per-stripe epilogues; band-0 epilogues issue before the
    band-2 prefetch so PSUM banks recycle promptly on DVE.
"""

import numpy as np

_B, _D, _DOUT = 4096, 4096, 4096
_NCORES = 8
_R, _C = 2, 4              # grid: 2 token-halves x 4 out_feature-quarters
_BPER = _B // _R           # 2048 tokens per core
_OPER = _DOUT // _C        # 1024 out features per core
_MAGIC = 12582912.0        # 1.5 * 2**23
_P = 128

_nc_cache = {}

# Chunk layouts (k-tiles per chunk). First chunks are small to fill the
# quantize pipeline fast; later chunks are fat to amortize triggers.
_WCHUNKS = [1, 1, 2] + [4] * 7           # 32 k-tiles, W chunks
_XCHUNKS0 = [1, 1, 2] + [4] * 7          # 32 k-tiles, x band 0
_XCHUNKS = [4] * 8                       # 32 k-tiles, x bands 1-3


def _starts(chunks):
    s, out = 0, []
    for c in chunks:
        out.append(s)
        s += c
    return out


def _build(BPER, D, OPER):
    import concourse.mybir as mybir
    import concourse.tile as tile
    from concourse import bacc

    f32 = mybir.dt.float32
    f16 = mybir.dt.float16
    P = _P
    KT = D // P            # 32 contraction k-tiles
    NB = 4                 # x bands
    BBAND = BPER // NB     # 512 tokens per band
    SBT = BBAND // P       # 4 stripes per band
    NH = OPER // 512       # 2 o-halves (moving dim is 512 max)
    mult = mybir.AluOpType.mult
    add = mybir.AluOpType.add

    wstarts = _starts(_WCHUNKS)
    xstarts0 = _starts(_XCHUNKS0)
    xstarts = _starts(_XCHUNKS)
    # kt -> (chunk index, offset) maps
    wmap = {}
    for ci, (st0, cnt) in enumerate(zip(wstarts, _WCHUNKS)):
        for k in range(cnt):
            wmap[st0 + k] = (ci, k)
    xmap0, xmap = {}, {}
    for ci, (st0, cnt) in enumerate(zip(xstarts0, _XCHUNKS0)):
        for k in range(cnt):
            xmap0[st0 + k] = (ci, k)
    for ci, (st0, cnt) in enumerate(zip(xstarts, _XCHUNKS)):
        for k in range(cnt):
            xmap[st0 + k] = (ci, k)

    nc = bacc.Bacc("TRN2", target_bir_lowering=False, debug=False,
                   num_devices=_NCORES)
    xt_d = nc.dram_tensor("xt", [D, BPER], f32, kind="ExternalInput").ap()
    wt_d = nc.dram_tensor("wt", [D, OPER], f32, kind="ExternalInput").ap()
    b_d = nc.dram_tensor("b", [OPER], f32, kind="ExternalInput").ap()
    o_d = nc.dram_tensor("out", [BPER, OPER], f32, kind="ExternalOutput").ap()

    with tile.TileContext(nc) as tc:
        with (
            tc.tile_pool(name="const", bufs=1) as cpool,
            tc.tile_pool(name="wq", bufs=1) as wqpool,
            tc.tile_pool(name="xq", bufs=1) as xq0pool,
            tc.tile_pool(name="xqs", bufs=2) as xqpool,
            tc.tile_pool(name="wstg", bufs=2) as wstgpool,
            tc.tile_pool(name="xstg", bufs=2) as xstgpool,
            tc.tile_pool(name="mm", bufs=8, space="PSUM") as mmpool,
            tc.tile_pool(name="osb", bufs=3) as opool,
        ):
            bias_bc = cpool.tile([P, OPER], f32)
            # GpSimd software DGE: keeps the slow broadcast expansion
            # off the Sync HWDGE ring head (it delayed the first input
            # transfer by several us).
            nc.gpsimd.dma_start(bias_bc, b_d[None, :].to_broadcast((P, OPER)))

            # Stripe each input stream across the two HWDGE queues with
            # independent toggles (W starts on Sync, x on Scalar). A
            # single shared toggle correlates with the W/x call
            # alternation and degenerates to W-on-Sync / x-on-Scalar,
            # serializing each stream on one queue.
            def dma_in(stream, dst_ap, src_ap):
                # All input triggers on the Sync ring: the Scalar engine
                # runs the fp16 converts, and a conv waiting on DMA data
                # would head-of-line block any trigger queued behind it.
                nc.sync.dma_start(dst_ap, src_ap)

            wq = [wqpool.tile([P, cnt * OPER], f16, tag=f"wq{c}",
                              name=f"wq{c}")
                  for c, cnt in enumerate(_WCHUNKS)]

            def quant(dst16, src32):
                # fl32(fl32(100*v) + MAGIC) then -MAGIC, both on DVE
                # (rounds half-to-even onto the integer grid, emits
                # fp16). Keeping quantization entirely on DVE leaves the
                # Scalar engine as a pure DMA-trigger queue: an ACT op
                # waiting on DMA data would head-of-line block the
                # Scalar HWDGE ring (measured: x stream serialized
                # behind W quantization).
                nc.vector.tensor_scalar(src32, src32, 100.0, _MAGIC,
                                        mult, add)
                # fp16 convert on the Scalar engine (ACT): DVE saturates
                # if it carries both quant passes (~143 us measured),
                # and GpSimd tensor ops are ~20x too slow.
                nc.scalar.activation(dst16, src32,
                                     mybir.ActivationFunctionType.Copy,
                                     bias=-_MAGIC, scale=1.0)

            def load_w_chunk(c):
                cnt = _WCHUNKS[c]
                k0 = wstarts[c]
                st_full = wstgpool.tile([P, 4 * OPER], f32, tag="wst",
                                        name=f"wst{c}")
                st = st_full[:, :cnt * OPER]
                src = wt_d[k0 * P:(k0 + cnt) * P, :].rearrange(
                    "(k p) o -> p k o", p=P)
                dma_in("w", st.rearrange("p (k o) -> p k o", k=cnt), src)
                quant(wq[c], st)

            def load_x_chunk(band, c, cnt, k0, tiles):
                st_full = xstgpool.tile([P, 4 * BBAND], f32, tag="xst",
                                        name=f"xst{band}_{c}")
                st = st_full[:, :cnt * BBAND]
                src = xt_d[k0 * P:(k0 + cnt) * P,
                           band * BBAND:(band + 1) * BBAND].rearrange(
                    "(k p) b -> p k b", p=P)
                dma_in("x", st.rearrange("p (k b) -> p k b", k=cnt), src)
                if band == 0 and cnt != 4:
                    # Small head chunks get a tiny dedicated pool; band
                    # 0's fat chunks share the rotating xq tags (band 2
                    # reuses them after band 0's matmuls complete).
                    t = xq0pool.tile([P, cnt * BBAND], f16, tag=f"xq0_{c}",
                                     name=f"xq0_{c}")
                else:
                    tg = c - 3 if band == 0 else c
                    t = xqpool.tile([P, cnt * BBAND], f16, tag=f"xq{tg}",
                                    name=f"xq{band}_{c}")
                quant(t, st)
                tiles.append(t)

            def load_band(band):
                tiles = []
                for c, cnt in enumerate(_XCHUNKS):
                    load_x_chunk(band, c, cnt, xstarts[c], tiles)
                return tiles

            def mm(ps, xqt, xm, kt, bt, oh, start, stop):
                xc, xk = xm[kt]
                wc, wk = wmap[kt]
                xof = xk * BBAND + bt * P
                wof = wk * OPER + oh * 512
                nc.tensor.matmul(
                    ps,
                    xqt[xc][:, xof:xof + P],
                    wq[wc][:, wof:wof + 512],
                    start=start, stop=stop)

            def epilogue_bt(band, bt, ps_pair):
                ob = opool.tile([P, OPER], f32, tag="ob",
                                name=f"ob{band}_{bt}")
                for oh in range(NH):
                    nc.vector.scalar_tensor_tensor(
                        ob[:, oh * 512:(oh + 1) * 512], ps_pair[oh], 1e-4,
                        bias_bc[:, oh * 512:(oh + 1) * 512], mult, add)
                row = (band * SBT + bt) * P
                nc.scalar.dma_start(o_d[row:row + P, :], ob)

            # Ramp: issue W chunk c and band-0 x chunk c back-to-back
            # (they cover the same k-tiles), alternating HWDGE queues.
            xq_tiles = [None] * NB
            xq_tiles[0] = []
            for c in range(len(_WCHUNKS)):
                load_w_chunk(c)
                load_x_chunk(0, c, _XCHUNKS0[c], xstarts0[c], xq_tiles[0])
            xq_tiles[1] = load_band(1)

            chains = [(bt, oh) for bt in range(SBT) for oh in range(NH)]

            # Band 0: kt-outer across all 8 PSUM banks (DMA-paced ramp).
            ps0 = [mmpool.tile([P, 512], f32, tag="ps", name=f"ps0_{j}")
                   for j in range(len(chains))]
            for kt in range(KT):
                for j, (bt, oh) in enumerate(chains):
                    mm(ps0[j], xq_tiles[0], xmap0, kt, bt, oh,
                       start=(kt == 0), stop=(kt == KT - 1))
            for bt in range(SBT):
                epilogue_bt(0, bt, ps0[bt * NH:(bt + 1) * NH])
            xq_tiles[2] = load_band(2)

            # Band 1 kt-outer too: its x chunks are still streaming in
            # behind W + band 0 when it starts, and chain-sequential
            # order stalled on the last chunks (measured ~8 us).
            ps1 = [mmpool.tile([P, 512], f32, tag="ps", name=f"ps1_{j}")
                   for j in range(len(chains))]
            for kt in range(KT):
                for j, (bt, oh) in enumerate(chains):
                    mm(ps1[j], xq_tiles[1], xmap, kt, bt, oh,
                       start=(kt == 0), stop=(kt == KT - 1))
            for bt in range(SBT):
                epilogue_bt(1, bt, ps1[bt * NH:(bt + 1) * NH])
            xq_tiles[3] = load_band(3)

            # Bands 2-3: fully prefetched, chain-sequential with
            # per-stripe epilogues (staggers the tail).
            for band in range(2, NB):
                for bt in range(SBT):
                    ps_pair = []
                    for oh in range(NH):
                        ps = mmpool.tile([P, 512], f32, tag="ps",
                                         name=f"ps{band}_{bt}_{oh}")
                        for kt in range(KT):
                            mm(ps, xq_tiles[band], xmap, kt, bt, oh,
                               start=(kt == 0), stop=(kt == KT - 1))
                        ps_pair.append(ps)
                    epilogue_bt(band, bt, ps_pair)

    nc.compile()
    return nc


def _get_nc(BPER=_BPER, D=_D, OPER=_OPER):
    key = (BPER, D, OPER)
    if key not in _nc_cache:
        _nc_cache[key] = _build(BPER, D, OPER)
    return _nc_cache[key]


def _make_in_maps(x, W, b):
    xt = [np.ascontiguousarray(x[r * _BPER:(r + 1) * _BPER, :].T)
          for r in range(_R)]
    wt = [np.ascontiguousarray(W[c * _OPER:(c + 1) * _OPER, :].T)
          for c in range(_C)]
    bs = [np.ascontiguousarray(b[c * _OPER:(c + 1) * _OPER])
          for c in range(_C)]
    in_maps = []
    for r in range(_R):
        for c in range(_C):
            in_maps.append({"xt": xt[r], "wt": wt[c], "b": bs[c]})
    return in_maps


def _assemble(blocks):
    return np.block([[blocks[r * _C + c] for c in range(_C)]
                     for r in range(_R)])


def _run(x, W, b, trace=False):
    from concourse.bass_utils import run_bass_kernel_spmd

    nc = _get_nc()
    in_maps = _make_in_maps(x, W, b)
    res = run_bass_kernel_spmd(nc, in_maps, core_ids=list(range(_NCORES)),
                               trace=trace)
    out = _assemble([res.results[c]["out"] for c in range(_NCORES)])
    return out, res


def kernel(x=None, W=None, b=None):
    x = np.ascontiguousarray(np.asarray(x, dtype=np.float32))
    W = np.ascontiguousarray(np.asarray(W, dtype=np.float32))
    b = np.ascontiguousarray(np.asarray(b, dtype=np.float32))
    out, _ = _run(x, W, b, trace=False)
    return out


# revision 28
# speedup vs baseline: 5.2684x; 1.0654x over previous
"""BSI quantized linear kernel for Trainium2 (8 NeuronCores, SPMD).

Computes out = round(x*100)/100 @ (round(W*100)/100).T + b for
x [4096, 4096] f32, W [4096, 4096] f32, b [4096] f32.

Sharding: 2x4 grid. x is sharded over the token dim into 2 halves;
W and b over out_features into 4 quarters. Core (r, c) = id r*4+c
computes out[r*2048:(r+1)*2048, c*1024:(c+1)*1024]; the host
assembles the 8 blocks.

Inputs are handed to each core pre-transposed (x^T and W^T slices —
host-side numpy layout change only, values bit-identical), so both
matmul operands arrive with the contraction dim on partitions and the
PE does no transposes: its only work is 1024 chained 128x512 fp16
matmuls (216 ns issue-to-issue measured => ~221 us at full rate).

Quantization (exact): DVE computes fl32(100*v) + 1.5*2^23 (f32 magic
constant => round-half-to-even onto the integer grid, matching
jnp.round bitwise), ACT subtracts the magic and emits fp16 (integers
|.|<=~550 are exact in fp16). PSUM accumulates exact integer dots in
f32; epilogue computes 1e-4*psum + bias in one DVE op per o-half.

Scheduling notes (from traces of earlier revisions):
  - Small per-kt DMAs serialize on HWDGE trigger processing (~230
    GB/s effective): load multi-kt chunks with one trigger each.
  - 3D SBUF tiles cost ~2x on ACT and +15% on matmul operands: all
    compute tiles are 2D; only the DMA *access patterns* are 3D
    (dst tile viewed p (k o) -> p k o, src DRAM (k p) o -> p k o).
  - TRN2 has two HWDGE queues (Sync and Scalar/Activation): input
    chunks alternate between them so both stream concurrently during
    the W+band-0 ramp (a single queue toped out at ~290-340 GB/s).
  - The first chunks are 1 k-tile so the DMA->DVE->ACT pipeline fills
    fast and the first matmul starts ~10 us earlier.
  - Output DMAs also go on the Scalar queue (they trail the ramp).
  - Band 0 issues matmuls kt-outer across all 8 PSUM banks to ride
    the DMA ramp; bands 1-3 are prefetched and run chain-sequential
    with per-stripe epilogues; band-0 epilogues issue before the
    band-2 prefetch so PSUM banks recycle promptly on DVE.
"""

import numpy as np

_B, _D, _DOUT = 4096, 4096, 4096
_NCORES = 8
_R, _C = 2, 4              # grid: 2 token-halves x 4 out_feature-quarters
_BPER = _B // _R           # 2048 tokens per core
_OPER = _DOUT // _C        # 1024 out features per core
_MAGIC = 12582912.0        # 1.5 * 2**23
_P = 128

_nc_cache = {}

# Chunk layouts (k-tiles per chunk). First chunks are small to fill the
# quantize pipeline fast; later chunks are fat to amortize triggers.
_WCHUNKS = [1, 1, 2] + [4] * 7           # 32 k-tiles, W chunks
_XCHUNKS0 = [1, 1, 2] + [4] * 7          # 32 k-tiles, x band 0
_XCHUNKS = [4] * 8                       # 32 k-tiles, x bands 1-3


def _starts(chunks):
    s, out = 0, []
    for c in chunks:
        out.append(s)
        s += c
    return out


def _build(BPER, D, OPER):
    import concourse.mybir as mybir
    import concourse.tile as tile
    from concourse import bacc

    f32 = mybir.dt.float32
    f16 = mybir.dt.float16
    P = _P
    KT = D // P            # 32 contraction k-tiles
    NB = 4                 # x bands
    BBAND = BPER // NB     # 512 tokens per band
    SBT = BBAND // P       # 4 stripes per band
    NH = OPER // 512       # 2 o-halves (moving dim is 512 max)
    mult = mybir.AluOpType.mult
    add = mybir.AluOpType.add

    wstarts = _starts(_WCHUNKS)
    xstarts0 = _starts(_XCHUNKS0)
    xstarts = _starts(_XCHUNKS)
    # kt -> (chunk index, offset) maps
    wmap = {}
    for ci, (st0, cnt) in enumerate(zip(wstarts, _WCHUNKS)):
        for k in range(cnt):
            wmap[st0 + k] = (ci, k)
    xmap0, xmap = {}, {}
    for ci, (st0, cnt) in enumerate(zip(xstarts0, _XCHUNKS0)):
        for k in range(cnt):
            xmap0[st0 + k] = (ci, k)
    for ci, (st0, cnt) in enumerate(zip(xstarts, _XCHUNKS)):
        for k in range(cnt):
            xmap[st0 + k] = (ci, k)

    nc = bacc.Bacc("TRN2", target_bir_lowering=False, debug=False,
                   num_devices=_NCORES)
    xt_d = nc.dram_tensor("xt", [D, BPER], f32, kind="ExternalInput").ap()
    wt_d = nc.dram_tensor("wt", [D, OPER], f32, kind="ExternalInput").ap()
    b_d = nc.dram_tensor("b", [OPER], f32, kind="ExternalInput").ap()
    o_d = nc.dram_tensor("out", [BPER, OPER], f32, kind="ExternalOutput").ap()

    with tile.TileContext(nc) as tc:
        with (
            tc.tile_pool(name="const", bufs=1) as cpool,
            tc.tile_pool(name="wq", bufs=1) as wqpool,
            tc.tile_pool(name="xq", bufs=1) as xq0pool,
            tc.tile_pool(name="xqs", bufs=2) as xqpool,
            tc.tile_pool(name="wstg", bufs=2) as wstgpool,
            tc.tile_pool(name="xstg", bufs=2) as xstgpool,
            tc.tile_pool(name="mm", bufs=8, space="PSUM") as mmpool,
            tc.tile_pool(name="osb", bufs=3) as opool,
        ):
            bias_bc = cpool.tile([P, OPER], f32)
            # GpSimd software DGE: keeps the slow broadcast expansion
            # off the Sync HWDGE ring head (it delayed the first input
            # transfer by several us).
            nc.gpsimd.dma_start(bias_bc, b_d[None, :].to_broadcast((P, OPER)))

            # Stripe each input stream across the two HWDGE queues with
            # independent toggles (W starts on Sync, x on Scalar). A
            # single shared toggle correlates with the W/x call
            # alternation and degenerates to W-on-Sync / x-on-Scalar,
            # serializing each stream on one queue.
            _qtog = {"w": 0, "x": 1}

            def dma_in(stream, dst_ap, src_ap):
                eng = nc.sync if _qtog[stream] == 0 else nc.scalar
                _qtog[stream] ^= 1
                eng.dma_start(dst_ap, src_ap)

            wq = [wqpool.tile([P, cnt * OPER], f16, tag=f"wq{c}",
                              name=f"wq{c}")
                  for c, cnt in enumerate(_WCHUNKS)]

            def quant(dst16, src32):
                # fl32(fl32(100*v) + MAGIC) then -MAGIC, both on DVE
                # (rounds half-to-even onto the integer grid, emits
                # fp16). Keeping quantization entirely on DVE leaves the
                # Scalar engine as a pure DMA-trigger queue: an ACT op
                # waiting on DMA data would head-of-line block the
                # Scalar HWDGE ring (measured: x stream serialized
                # behind W quantization).
                nc.vector.tensor_scalar(src32, src32, 100.0, _MAGIC,
                                        mult, add)
                nc.vector.tensor_scalar_add(dst16, src32, -_MAGIC)

            def load_w_chunk(c):
                cnt = _WCHUNKS[c]
                k0 = wstarts[c]
                st_full = wstgpool.tile([P, 4 * OPER], f32, tag="wst",
                                        name=f"wst{c}")
                st = st_full[:, :cnt * OPER]
                src = wt_d[k0 * P:(k0 + cnt) * P, :].rearrange(
                    "(k p) o -> p k o", p=P)
                dma_in("w", st.rearrange("p (k o) -> p k o", k=cnt), src)
                quant(wq[c], st)

            def load_x_chunk(band, c, cnt, k0, tiles):
                st_full = xstgpool.tile([P, 4 * BBAND], f32, tag="xst",
                                        name=f"xst{band}_{c}")
                st = st_full[:, :cnt * BBAND]
                src = xt_d[k0 * P:(k0 + cnt) * P,
                           band * BBAND:(band + 1) * BBAND].rearrange(
                    "(k p) b -> p k b", p=P)
                dma_in("x", st.rearrange("p (k b) -> p k b", k=cnt), src)
                if band == 0 and cnt != 4:
                    # Small head chunks get a tiny dedicated pool; band
                    # 0's fat chunks share the rotating xq tags (band 2
                    # reuses them after band 0's matmuls complete).
                    t = xq0pool.tile([P, cnt * BBAND], f16, tag=f"xq0_{c}",
                                     name=f"xq0_{c}")
                else:
                    tg = c - 3 if band == 0 else c
                    t = xqpool.tile([P, cnt * BBAND], f16, tag=f"xq{tg}",
                                    name=f"xq{band}_{c}")
                quant(t, st)
                tiles.append(t)

            def load_band(band):
                tiles = []
                for c, cnt in enumerate(_XCHUNKS):
                    load_x_chunk(band, c, cnt, xstarts[c], tiles)
                return tiles

            def mm(ps, xqt, xm, kt, bt, oh, start, stop):
                xc, xk = xm[kt]
                wc, wk = wmap[kt]
                xof = xk * BBAND + bt * P
                wof = wk * OPER + oh * 512
                nc.tensor.matmul(
                    ps,
                    xqt[xc][:, xof:xof + P],
                    wq[wc][:, wof:wof + 512],
                    start=start, stop=stop)

            def epilogue_bt(band, bt, ps_pair):
                ob = opool.tile([P, OPER], f32, tag="ob",
                                name=f"ob{band}_{bt}")
                for oh in range(NH):
                    nc.vector.scalar_tensor_tensor(
                        ob[:, oh * 512:(oh + 1) * 512], ps_pair[oh], 1e-4,
                        bias_bc[:, oh * 512:(oh + 1) * 512], mult, add)
                row = (band * SBT + bt) * P
                nc.scalar.dma_start(o_d[row:row + P, :], ob)

            # Ramp: issue W chunk c and band-0 x chunk c back-to-back
            # (they cover the same k-tiles), alternating HWDGE queues.
            xq_tiles = [None] * NB
            xq_tiles[0] = []
            for c in range(len(_WCHUNKS)):
                load_w_chunk(c)
                load_x_chunk(0, c, _XCHUNKS0[c], xstarts0[c], xq_tiles[0])
            xq_tiles[1] = load_band(1)

            chains = [(bt, oh) for bt in range(SBT) for oh in range(NH)]

            # Band 0: kt-outer across all 8 PSUM banks (DMA-paced ramp).
            ps0 = [mmpool.tile([P, 512], f32, tag="ps", name=f"ps0_{j}")
                   for j in range(len(chains))]
            for kt in range(KT):
                for j, (bt, oh) in enumerate(chains):
                    mm(ps0[j], xq_tiles[0], xmap0, kt, bt, oh,
                       start=(kt == 0), stop=(kt == KT - 1))
            for bt in range(SBT):
                epilogue_bt(0, bt, ps0[bt * NH:(bt + 1) * NH])
            xq_tiles[2] = load_band(2)

            # Bands 1-3: fully prefetched, chain-sequential with
            # per-stripe epilogues.
            for band in range(1, NB):
                for bt in range(SBT):
                    ps_pair = []
                    for oh in range(NH):
                        ps = mmpool.tile([P, 512], f32, tag="ps",
                                         name=f"ps{band}_{bt}_{oh}")
                        for kt in range(KT):
                            mm(ps, xq_tiles[band], xmap, kt, bt, oh,
                               start=(kt == 0), stop=(kt == KT - 1))
                        ps_pair.append(ps)
                    epilogue_bt(band, bt, ps_pair)
                    if band + 2 < NB and bt == 0:
                        xq_tiles[band + 2] = load_band(band + 2)

    nc.compile()
    return nc


def _get_nc(BPER=_BPER, D=_D, OPER=_OPER):
    key = (BPER, D, OPER)
    if key not in _nc_cache:
        _nc_cache[key] = _build(BPER, D, OPER)
    return _nc_cache[key]


def _make_in_maps(x, W, b):
    xt = [np.ascontiguousarray(x[r * _BPER:(r + 1) * _BPER, :].T)
          for r in range(_R)]
    wt = [np.ascontiguousarray(W[c * _OPER:(c + 1) * _OPER, :].T)
          for c in range(_C)]
    bs = [np.ascontiguousarray(b[c * _OPER:(c + 1) * _OPER])
          for c in range(_C)]
    in_maps = []
    for r in range(_R):
        for c in range(_C):
            in_maps.append({"xt": xt[r], "wt": wt[c], "b": bs[c]})
    return in_maps


def _assemble(blocks):
    return np.block([[blocks[r * _C + c] for c in range(_C)]
                     for r in range(_R)])


def _run(x, W, b, trace=False):
    from concourse.bass_utils import run_bass_kernel_spmd

    nc = _get_nc()
    in_maps = _make_in_maps(x, W, b)
    res = run_bass_kernel_spmd(nc, in_maps, core_ids=list(range(_NCORES)),
                               trace=trace)
    out = _assemble([res.results[c]["out"] for c in range(_NCORES)])
    return out, res


def kernel(x=None, W=None, b=None):
    x = np.ascontiguousarray(np.asarray(x, dtype=np.float32))
    W = np.ascontiguousarray(np.asarray(W, dtype=np.float32))
    b = np.ascontiguousarray(np.asarray(b, dtype=np.float32))
    out, _ = _run(x, W, b, trace=False)
    return out
